# revision 1
# baseline (speedup 1.0000x reference)
"""Trainium2 Bass kernel for nn_Decoder_1D_Matryoshka (12-layer masked decoder).

Strategy: data-parallel over B (16 samples -> 8 cores x 2), with the two
samples per core merged into one token axis so every weight slab is loaded
once per layer. Residual kept transposed (features on partitions). The mask
structure is exploited two ways: (1) latent queries never attend mask keys,
so that score quadrant is skipped; (2) tokens >= M+Ni are dead, so samples
with Ni<=128 run with a 384-token sequence. Host pairs one long (512) and
one short (384) sample per core when possible (fallback: 512+512 program).
q/k projections + layernorm statistics run in fp8e4 DoubleRow (2x tensor
throughput); v/proj/ff stay bf16 (fp8 there fails the accuracy budget).
"""

import numpy as np
import ml_dtypes

B, NLAT, DIN = 16, 256, 32
D, H, NL, VAE = 1024, 16, 12, 16
M = 256
L = 512
DH = 64
NCORES = 8
BPC = 2
P = 128
KT = D // P       # 8 feature bands
FF = 4 * D

BF16 = ml_dtypes.bfloat16
E4 = ml_dtypes.float8_e4m3
F8MAX = np.float32(240.0)

# param-tile column layout (one [128, 112] f32 DMA per layer)
PC_SQ = 0      # 8: q dequant scale per band
PC_SK = 8      # 8
PC_BQ = 16     # 8: q bias (pre-scaled)
PC_BK = 24     # 8
PC_BP = 32     # 8: proj bias
PC_B1 = 40     # 32: fc1 bias
PC_B2 = 72     # 8: fc2 bias
PC_G1 = 80     # 8
PC_BL1 = 88    # 8
PC_G2 = 96     # 8
PC_BL2 = 104   # 8
PCOLS = 112


def _layernorm_np(x, g, b, eps=1e-5):
    mu = x.mean(-1, keepdims=True)
    var = ((x - mu) ** 2).mean(-1, keepdims=True)
    return (x - mu) / np.sqrt(var + eps) * g + b


def _build_mask_T(ni):
    # additive mask, transposed: maskT[b, j, i] = bias[b, i, j]; -30 for masked
    r = np.arange(L)[:, None]
    c = np.arange(L)[None, :]
    Ni = ni[:, None, None].astype(np.int64)
    ok = (r < M) & (c < M)
    ok = ok | ((r < M) & (c >= M) & (c < M + Ni))
    ok = ok | ((r >= M) & (r < M + Ni) & (c >= M) & (c <= r))
    ok = ok | ((r >= M + Ni) & (c >= M) & (c < M + Ni))
    bias = np.where(ok, np.float32(1.0), np.float32(0.0))
    return bias.transpose(0, 2, 1).copy()


def _quant_cols(w):
    """per-output-column absmax quantize to TRN e4m3 (max 240)."""
    amax = np.abs(w).max(axis=0, keepdims=True)
    s = np.where(amax > 0, amax / F8MAX, np.float32(1.0)).astype(np.float32)
    w8 = np.clip(w / s, -F8MAX, F8MAX).astype(E4)
    return w8, s[0]


def _host_prep(inputs):
    """Returns (in_maps, LB, perm) where perm[core] = (sampleA, sampleB)."""
    f32 = np.float32
    lat = inputs["latents"].astype(f32)
    x_lat = lat.reshape(B * NLAT, DIN) @ inputs["input_w"].astype(f32)
    x_lat = x_lat.reshape(B, NLAT, D) + inputs["input_b"][None, None, :]
    x_lat = x_lat + inputs["latents_pos_embed"][None, :, :]
    mt = inputs["mask_tokens"].reshape(1, 1, D) + inputs["pos_embed_full"]
    mt = np.broadcast_to(mt, (B, M, D))
    x = np.concatenate([mt, x_lat], axis=1)                       # (B, L, D)
    x = _layernorm_np(x, inputs["ln_pre_g"], inputs["ln_pre_b"]).astype(f32)

    ni = np.asarray(inputs["num_activated"]).astype(np.int64)
    maskT = _build_mask_T(ni)                                     # (B,L,L) f32

    # ---- sample pairing: big samples first, small last ----
    leff = np.where(ni <= M - P, 384, 512)   # Ni<=128 -> 384-token class
    order = np.argsort(-leff, kind="stable")
    LB = 384 if (leff[order[NCORES:]] == 384).all() else 512
    perm = [(int(order[i]), int(order[2 * NCORES - 1 - i])) for i in range(NCORES)]
    latB = LB - M

    # ---- weights ----
    scale = np.float32(DH ** -0.5)
    qkv = inputs["qkv_w"].astype(f32)
    qkvb = inputs["qkv_b"].astype(f32)
    wqk8 = np.zeros((NL, D, 2 * D), E4)
    params = np.zeros((NL, P, PCOLS), f32)
    for l in range(NL):
        wqk = np.concatenate([qkv[l][:, :D] * scale, qkv[l][:, D:2 * D]], axis=1)
        w8, s = _quant_cols(wqk)
        wqk8[l] = w8
        params[l, :, PC_SQ:PC_SK + 8] = s.reshape(16, P).T
    bq = qkvb[:, :D] * scale
    bk = qkvb[:, D:2 * D]
    params[:, :, PC_BQ:PC_BQ + 8] = bq.reshape(NL, 8, P).transpose(0, 2, 1)
    params[:, :, PC_BK:PC_BK + 8] = bk.reshape(NL, 8, P).transpose(0, 2, 1)
    params[:, :, PC_BP:PC_BP + 8] = inputs["proj_b"].reshape(NL, 8, P).transpose(0, 2, 1)
    params[:, :, PC_B1:PC_B1 + 32] = inputs["fc1_b"].reshape(NL, 32, P).transpose(0, 2, 1)
    params[:, :, PC_B2:PC_B2 + 8] = inputs["fc2_b"].reshape(NL, 8, P).transpose(0, 2, 1)
    params[:, :, PC_G1:PC_G1 + 8] = inputs["ln1_g"].reshape(NL, 8, P).transpose(0, 2, 1)
    params[:, :, PC_BL1:PC_BL1 + 8] = inputs["ln1_b"].reshape(NL, 8, P).transpose(0, 2, 1)
    params[:, :, PC_G2:PC_G2 + 8] = inputs["ln2_g"].reshape(NL, 8, P).transpose(0, 2, 1)
    params[:, :, PC_BL2:PC_BL2 + 8] = inputs["ln2_b"].reshape(NL, 8, P).transpose(0, 2, 1)

    bvrow = qkvb[:, 2 * D:].reshape(NL, 1, D).astype(f32)

    postparams = np.zeros((P, 16), f32)
    postparams[:, 0:8] = inputs["ln_post_g"].reshape(8, P).T
    postparams[:, 8:16] = inputs["ln_post_b"].reshape(8, P).T

    # repack q/k weights so each 128-col n-tile has its 8 contraction planes
    # adjacent in SBUF (dual-fp8 LdWeights needs contiguous plane pairs):
    # [l, row(po*128+pi), n*128+c] -> [l, n, pi, po, c]
    wqk8_r = np.ascontiguousarray(
        wqk8.reshape(NL, KT, P, 16, P).transpose(0, 3, 2, 1, 4))

    static = {
        "wqk8": wqk8_r,
        "wv": np.ascontiguousarray(qkv[:, :, 2 * D:].astype(BF16)),
        "wproj": np.ascontiguousarray(inputs["proj_w"].astype(BF16)),
        "w1": np.ascontiguousarray(inputs["fc1_w"].astype(BF16)),
        "w2": np.ascontiguousarray(inputs["fc2_w"].astype(BF16)),
        "params": params,
        "bvrow": np.ascontiguousarray(bvrow),
        "postparams": postparams,
        "wout": np.ascontiguousarray(
            inputs["out_w"].astype(BF16).reshape(KT, P, VAE)),
        "bout": inputs["out_b"].astype(f32).reshape(VAE, 1),
    }

    in_maps = []
    for c in range(NCORES):
        sa, sb = perm[c]
        xc = np.concatenate([x[sa], x[sb, :LB]], axis=0)          # (T, D)
        x0T = np.ascontiguousarray(xc.T).astype(f32)              # (D, T)
        mA = maskT[sa][2 * P:4 * P, :].astype(BF16)               # (256, 512)
        mB = maskT[sb][2 * P:2 * P + latB, :LB].astype(BF16)      # (latB, LB)
        m = {
            "x0T": x0T,
            "maskA": np.ascontiguousarray(mA),
            "maskB": np.ascontiguousarray(mB),
        }
        m.update(static)
        in_maps.append(m)
    return in_maps, LB, perm


_PROGS = {}
_PROG = None


def _build_bass(LB):
    import concourse.bass as bass
    import concourse.bacc as bacc
    import concourse.tile as tile
    import concourse.mybir as mybir
    from contextlib import ExitStack

    f32 = mybir.dt.float32
    bf16 = mybir.dt.bfloat16
    fp8 = mybir.dt.float8e4
    AF = mybir.ActivationFunctionType
    OP = mybir.AluOpType
    DR = mybir.MatmulPerfMode.DoubleRow

    T = 512 + LB
    latB = LB - M                 # latent tokens of sample B (128 or 256)
    NLATB = latB // P             # latent key chunks of B (1 or 2)
    NMB = T // P                  # 128-token chunks (7 or 8)
    CH256 = [(c, min(c + 256, T)) for c in range(0, T, 256)]
    CH512 = [(0, 512), (512, T)]

    nc = bacc.Bacc(None, target_bir_lowering=False, debug=False)
    dp = nc.declare_dram_parameter
    x0T = dp("x0T", [D, T], f32, isOutput=False)
    maskA = dp("maskA", [2 * P, 512], bf16, isOutput=False)
    maskB = dp("maskB", [latB, LB], bf16, isOutput=False)
    wqk8_d = dp("wqk8", [NL, 16, P, KT, P], fp8, isOutput=False)
    wv_d = dp("wv", [NL, D, D], bf16, isOutput=False)
    wproj_d = dp("wproj", [NL, D, D], bf16, isOutput=False)
    w1_d = dp("w1", [NL, D, FF], bf16, isOutput=False)
    w2_d = dp("w2", [NL, FF, D], bf16, isOutput=False)
    params_d = dp("params", [NL, P, PCOLS], f32, isOutput=False)
    bvrow_d = dp("bvrow", [NL, 1, D], f32, isOutput=False)
    postparams = dp("postparams", [P, 16], f32, isOutput=False)
    wout_d = dp("wout", [KT, P, VAE], bf16, isOutput=False)
    bout_d = dp("bout", [VAE, 1], f32, isOutput=False)
    out_d = dp("out", [BPC, M, VAE], f32, isOutput=True)

    def rchunks(c0, c1):
        out = []
        c = c0
        while c < c1:
            out.append((c, min(c + 256, c1)))
            c = out[-1][1]
        return out

    with ExitStack() as ctx:
        tc = ctx.enter_context(tile.TileContext(nc))
        # ---- pools ----
        persist = ctx.enter_context(tc.tile_pool(name="persist", bufs=1))
        wqkp = ctx.enter_context(tc.tile_pool(name="wqk", bufs=6))
        wbfp = ctx.enter_context(tc.tile_pool(name="wbf", bufs=4))
        hpool = ctx.enter_context(tc.tile_pool(name="h", bufs=1))
        tpool = ctx.enter_context(tc.tile_pool(name="lnt", bufs=2))
        h8pool = ctx.enter_context(tc.tile_pool(name="h8", bufs=1))
        x8pool = ctx.enter_context(tc.tile_pool(name="x8", bufs=2))
        gpool = ctx.enter_context(tc.tile_pool(name="g", bufs=1))
        epool = ctx.enter_context(tc.tile_pool(name="e", bufs=2))
        spool = ctx.enter_context(tc.tile_pool(name="stats", bufs=4))
        rpool = ctx.enter_context(tc.tile_pool(name="rsb", bufs=2))
        ppool = ctx.enter_context(tc.tile_pool(name="lparams", bufs=2))
        ypool = ctx.enter_context(tc.tile_pool(name="yout", bufs=2))
        psp = ctx.enter_context(tc.tile_pool(name="psp", bufs=7, space="PSUM"))

        def bank(name):
            return psp.tile([P, 512], f32, tag="bank", name=name)

        # ---- persistent tiles ----
        xt = [persist.tile([P, T], f32, tag=f"x{k}", name=f"x{k}") for k in range(KT)]
        qt = [persist.tile([P, T], fp8, tag=f"q{n}", name=f"q{n}") for n in range(KT)]
        kt_ = [persist.tile([P, T], fp8, tag=f"k{n}", name=f"k{n}") for n in range(KT)]
        v2A = persist.tile([P, 4, D], bf16, tag="v2A", name="v2A")
        v2B = persist.tile([P, NMB - 4, D], bf16, tag="v2B", name="v2B")
        attnT = [persist.tile([P, T], bf16, tag=f"at{k}", name=f"at{k}") for k in range(KT)]
        mskA = persist.tile([P, 2, 512], bf16, tag="mskA", name="mskA")
        mskB = persist.tile([P, NLATB, LB], bf16, tag="mskB", name="mskB")
        btile = persist.tile([P, D], f32, tag="btile", name="btile")
        ones8 = persist.tile([P, 2, P], fp8, tag="ones8", name="ones8")
        ones_b = persist.tile([P, 1], bf16, tag="ones_b", name="ones_b")
        ones_row = persist.tile([1, P], bf16, tag="ones_row", name="ones_row")
        woutt = persist.tile([P, KT * VAE], bf16, tag="wo", name="wo")
        boutt = persist.tile([VAE, 1], f32, tag="bout", name="bout")
        postt = persist.tile([P, 16], f32, tag="post", name="post")
        epst = persist.tile([1, 1], f32, tag="eps", name="eps")
        zerot = persist.tile([P, 1], f32, tag="zerot", name="zerot")
        neg2 = persist.tile([P, 1], f32, tag="neg2", name="neg2")

        nc.vector.memset(epst, 1e-5)
        nc.vector.memset(zerot, 0.0)
        nc.vector.memset(neg2, -2.0)
        nc.vector.memset(ones8, 1.0)
        nc.vector.memset(ones_b, 1.0)
        nc.vector.memset(ones_row, 1.0)

        for k in range(KT):
            nc.sync.dma_start(out=xt[k], in_=x0T[k * P:(k + 1) * P, :])
            nc.sync.dma_start(out=woutt[:, k * VAE:(k + 1) * VAE], in_=wout_d[k])
        nc.sync.dma_start(out=mskA, in_=maskA.rearrange("(j pi) f -> pi j f", pi=P))
        nc.sync.dma_start(out=mskB, in_=maskB.rearrange("(j pi) f -> pi j f", pi=P))
        nc.sync.dma_start(out=boutt, in_=bout_d[:, :])
        nc.sync.dma_start(out=postt, in_=postparams[:, :])

        def emit_ln(c0, c1, grow, nbrow, out_tiles, out8, ow):
            """LN over features of xt[:, c0:c1] -> out_tiles[k][:, ow:ow+w]
            (bf16), optionally also into out8 [P,8,T] fp8 (gpsimd copy).
            Stats via fp8 DoubleRow pairs. Normalization: two ones-broadcast
            matmuls (prs, pms = mu*rstd broadcast), then per band
            t = x*prs - pms (2 vector ops), h = t*g + b (gpsimd)."""
            w = c1 - c0
            rcs = rchunks(c0, c1)
            psxs = [bank("psx") for _ in rcs]
            psxxs = [bank("psxx") for _ in rcs]
            for jp in range(4):
                xq = x8pool.tile([P, 2, 512], fp8, tag="xq", name="xq")
                xsq = x8pool.tile([P, 2, 512], fp8, tag="xsq", name="xsq")
                for i in range(2):
                    nc.vector.tensor_scalar_mul(
                        xq[:, i, :w], xt[2 * jp + i][:, c0:c1], 0.25)
                nc.vector.tensor_mul(xsq[:, :, :w], xq[:, :, :w], xq[:, :, :w])
                for ri, (r0, r1) in enumerate(rcs):
                    rw = r1 - r0
                    o = r0 - c0
                    nc.tensor.matmul(psxs[ri][:, :rw], ones8, xq[:, :, o:o + rw],
                                     start=(jp == 0), stop=(jp == 3), perf_mode=DR)
                    nc.tensor.matmul(psxxs[ri][:, :rw], ones8, xsq[:, :, o:o + rw],
                                     start=(jp == 0), stop=(jp == 3), perf_mode=DR)
            mu = spool.tile([1, 512], f32, tag="stat", name="mu")
            e2 = spool.tile([1, 512], f32, tag="stat", name="e2")
            rstd = spool.tile([1, 512], f32, tag="stat", name="rstd")
            musq = spool.tile([1, 512], f32, tag="stat", name="musq")
            mursb = spool.tile([1, 512], bf16, tag="statb", bufs=2, name="mursb")
            rstdb = spool.tile([1, 512], bf16, tag="statb", bufs=2, name="rstdb")
            for ri, (r0, r1) in enumerate(rcs):
                rw = r1 - r0
                o = r0 - c0
                nc.vector.tensor_scalar_mul(mu[:, o:o + rw], psxs[ri][0:1, :rw], 4.0 / D)
                nc.vector.tensor_scalar_mul(e2[:, o:o + rw], psxxs[ri][0:1, :rw], 16.0 / D)
            nc.vector.tensor_mul(musq[:, :w], mu[:, :w], mu[:, :w])
            nc.vector.tensor_sub(e2[:, :w], e2[:, :w], musq[:, :w])
            nc.scalar.activation(e2[:, :w], e2[:, :w], AF.Sqrt, bias=epst)
            nc.vector.reciprocal_approx_fast(out=rstd[:, :w], in_=e2[:, :w])
            nc.vector.tensor_mul(musq[:, :w], mu[:, :w], rstd[:, :w])
            nc.vector.tensor_copy(out=mursb[:, :w], in_=musq[:, :w])
            nc.vector.tensor_copy(out=rstdb[:, :w], in_=rstd[:, :w])
            pms = bank("pms")
            prs = bank("prs")
            nc.tensor.matmul(pms[:, :w], ones_row, mursb[:, :w], start=True, stop=True)
            nc.tensor.matmul(prs[:, :w], ones_row, rstdb[:, :w], start=True, stop=True)
            for k in range(KT):
                t = tpool.tile([P, 512], f32, tag="lnt", name="lnt")
                nc.vector.tensor_mul(t[:, :w], xt[k][:, c0:c1], prs[:, :w])
                nc.vector.tensor_sub(t[:, :w], t[:, :w], pms[:, :w])
                nc.gpsimd.tensor_scalar(
                    out=out_tiles[k][:, ow + 0:ow + w], in0=t[:, :w],
                    scalar1=grow[k], scalar2=nbrow[k],
                    op0=OP.mult, op1=OP.add)
                if out8 is not None:
                    nc.gpsimd.tensor_copy(out=out8[:, k, ow:ow + w],
                                          in_=out_tiles[k][:, ow:ow + w])

        def attn_sample(off, SL, nlat, msk, vt):
            """Attention for one sample at token offset off, seq len SL."""
            for hh in range(H):
                band = hh // 2
                po = (hh % 2) * DH
                q_ap = qt[band]
                k_ap = kt_[band]
                # mask-key chunks: only the 256 mask queries attend them.
                # both chunks packed in one psum bank (lazy zero-region),
                # single exp over 512 cols.
                e01 = epool.tile([P, 2, 256], bf16, tag="e01", name="e01")
                psS01 = bank("psS01")
                for j in range(2):
                    nc.tensor.matmul(psS01[:, j * 256:(j + 1) * 256],
                                     k_ap[po:po + DH, off + j * P:off + (j + 1) * P],
                                     q_ap[po:po + DH, off:off + 256],
                                     start=(j == 0), stop=(j == 1),
                                     skip_group_check=True)
                nc.scalar.activation(e01[:, :, :], psS01, AF.Exp, bias=neg2)
                # latent-key chunks: all SL queries, masked
                elat = []
                for j in range(nlat):
                    psS = bank("psSL")
                    nc.tensor.matmul(psS[:, :SL],
                                     k_ap[po:po + DH, off + (2 + j) * P:off + (3 + j) * P],
                                     q_ap[po:po + DH, off:off + SL],
                                     start=True, stop=True)
                    e = epool.tile([P, 512], bf16, tag=f"el{j}", name=f"el{j}")
                    nc.scalar.activation(e[:, :SL], psS[:, :SL], AF.Exp, bias=neg2)
                    nc.gpsimd.tensor_mul(e[:, :SL], e[:, :SL], msk[:, j, :SL])
                    elat.append(e)
                # mask queries (cols 0:256): all key chunks contribute
                nmm = 2 + nlat
                pss1 = bank("pss1")[0:1, :256]
                psO1 = bank("psO1")[0:DH, :256]
                i = 0
                for j in range(2):
                    nc.tensor.matmul(pss1, ones_b, e01[:, j, :],
                                     start=(i == 0), stop=(i == nmm - 1))
                    i += 1
                for j in range(nlat):
                    nc.tensor.matmul(pss1, ones_b, elat[j][:, :256],
                                     start=(i == 0), stop=(i == nmm - 1))
                    i += 1
                i = 0
                for j in range(2):
                    nc.tensor.matmul(psO1, vt[:, j, hh * DH:(hh + 1) * DH],
                                     e01[:, j, :], start=(i == 0), stop=(i == nmm - 1))
                    i += 1
                for j in range(nlat):
                    nc.tensor.matmul(psO1, vt[:, 2 + j, hh * DH:(hh + 1) * DH],
                                     elat[j][:, :256], start=(i == 0),
                                     stop=(i == nmm - 1))
                    i += 1
                rs1 = spool.tile([1, 256], f32, tag="rs", bufs=2, name="rs1")
                nc.vector.reciprocal_approx_fast(out=rs1, in_=pss1)
                rsb1 = rpool.tile([DH, 256], f32, tag="rsb", name="rsb1")
                nc.gpsimd.partition_broadcast(rsb1, rs1)
                nc.vector.tensor_mul(attnT[band][po:po + DH, off:off + 256],
                                     psO1, rsb1)
                # latent queries (cols 256:SL): latent keys only
                lw = SL - 256
                pss2 = bank("pss2")[0:1, :256]
                psO2 = bank("psO2")[0:DH, :256]
                for j in range(nlat):
                    nc.tensor.matmul(pss2[:, :lw], ones_b, elat[j][:, 256:SL],
                                     start=(j == 0), stop=(j == nlat - 1))
                for j in range(nlat):
                    nc.tensor.matmul(psO2[:, :lw],
                                     vt[:, 2 + j, hh * DH:(hh + 1) * DH],
                                     elat[j][:, 256:SL], start=(j == 0),
                                     stop=(j == nlat - 1))
                rs2 = spool.tile([1, 256], f32, tag="rs", bufs=2, name="rs2")
                nc.vector.reciprocal_approx_fast(out=rs2[:, :lw], in_=pss2[:, :lw])
                rsb2 = rpool.tile([DH, 256], f32, tag="rsb", name="rsb2")
                nc.gpsimd.partition_broadcast(rsb2[:, :lw], rs2[:, :lw])
                nc.vector.tensor_mul(attnT[band][po:po + DH, off + 256:off + SL],
                                     psO2[:, :lw], rsb2[:, :lw])

        for l in range(NL):
            pt = ppool.tile([P, PCOLS], f32, tag="pt", name="pt")
            bvr = ppool.tile([1, D], f32, tag="bvr", bufs=1, name="bvr")
            nc.sync.dma_start(out=pt, in_=params_d[l])
            nc.sync.dma_start(out=bvr, in_=bvrow_d[l])
            nc.gpsimd.partition_broadcast(btile, bvr)
            g1 = [pt[:, PC_G1 + k:PC_G1 + k + 1] for k in range(KT)]
            bl1 = [pt[:, PC_BL1 + k:PC_BL1 + k + 1] for k in range(KT)]
            g2 = [pt[:, PC_G2 + k:PC_G2 + k + 1] for k in range(KT)]
            bl2 = [pt[:, PC_BL2 + k:PC_BL2 + k + 1] for k in range(KT)]

            # ---------------- LN1 ----------------
            hb = [hpool.tile([P, T], bf16, tag=f"h{k}", name=f"h{k}") for k in range(KT)]
            h8 = h8pool.tile([P, KT, T], fp8, tag="h8", name="h8")
            for (c0, c1) in CH512:
                emit_ln(c0, c1, g1, bl1, hb, h8, c0)

            # ---------------- q/k (fp8 DoubleRow) ----------------
            for n in range(16):
                wqk = wqkp.tile([P, KT, P], fp8, tag="wqk", name="wqk")
                nc.sync.dma_start(out=wqk, in_=wqk8_d[l, n])
                if n < 8:
                    dst, sc, bc = qt[n], PC_SQ + n, PC_BQ + n
                else:
                    dst, sc, bc = kt_[n - 8], PC_SK + n - 8, PC_BK + n - 8
                for (C0, C1) in CH512:
                    ps = bank("psqk")
                    for ri, (r0, r1) in enumerate(rchunks(C0, C1)):
                        rw = r1 - r0
                        o = r0 - C0
                        for j in range(0, KT, 2):
                            nc.tensor.matmul(
                                ps[:, o:o + rw], wqk[:, j:j + 2, :],
                                h8[:, j:j + 2, r0:r1],
                                start=(ri == 0 and j == 0),
                                stop=(j == KT - 2), perf_mode=DR,
                                skip_group_check=True)
                    nc.vector.tensor_scalar(
                        out=dst[:, C0:C1], in0=ps[:, :C1 - C0],
                        scalar1=pt[:, sc:sc + 1], scalar2=pt[:, bc:bc + 1],
                        op0=OP.mult, op1=OP.add)

            # ---------------- v (bf16, h-stationary) ----------------
            for half in range(2):
                wv = wbfp.tile([P, KT, 512], bf16, tag="wbf", name="wv")
                nc.sync.dma_start(
                    out=wv,
                    in_=wv_d[l, :, half * 512:(half + 1) * 512].rearrange(
                        "(po pi) f -> pi po f", pi=P))
                for mb in range(NMB):
                    ps = bank("psv")
                    for k in range(KT):
                        nc.tensor.matmul(ps, hb[k][:, mb * P:(mb + 1) * P],
                                         wv[:, k, :], start=(k == 0), stop=(k == KT - 1))
                    dstv = v2A[:, mb, :] if mb < 4 else v2B[:, mb - 4, :]
                    nc.vector.tensor_add(
                        dstv[:, half * 512:(half + 1) * 512], ps,
                        btile[:, half * 512:(half + 1) * 512])

            # ---------------- attention ----------------
            attn_sample(0, 512, 2, mskA, v2A)
            attn_sample(512, LB, NLATB, mskB, v2B)

            # ---------------- proj (+residual) ----------------
            for half in range(2):
                wp = wbfp.tile([P, KT, 512], bf16, tag="wbf", name="wp")
                nc.sync.dma_start(
                    out=wp,
                    in_=wproj_d[l, :, half * 512:(half + 1) * 512].rearrange(
                        "(po pi) f -> pi po f", pi=P))
                for ni in range(4):
                    n = half * 4 + ni
                    for (c0, c1) in CH512:
                        w = c1 - c0
                        ps = bank("pspj")
                        for k in range(KT):
                            nc.tensor.matmul(ps[:, :w], wp[:, k, ni * P:(ni + 1) * P],
                                             attnT[k][:, c0:c1],
                                             start=(k == 0), stop=(k == KT - 1))
                        nc.vector.scalar_tensor_tensor(
                            out=xt[n][:, c0:c1], in0=ps[:, :w],
                            scalar=pt[:, PC_BP + n:PC_BP + n + 1],
                            in1=xt[n][:, c0:c1], op0=OP.add, op1=OP.add)

            # ---------------- LN2 ----------------
            hb = [hpool.tile([P, T], bf16, tag=f"h{k}", name=f"h{k}") for k in range(KT)]
            for (c0, c1) in CH512:
                emit_ln(c0, c1, g2, bl2, hb, None, c0)

            # ---------------- FF (4 quarters of d_ff) ----------------
            for cq in range(4):
                gt = [gpool.tile([P, T], bf16, tag=f"g{i}", name=f"g{i}")
                      for i in range(8)]
                for sl in range(2):
                    w1t = wbfp.tile([P, KT, 512], bf16, tag="wbf", name="w1t")
                    co = cq * 1024 + sl * 512
                    nc.sync.dma_start(
                        out=w1t,
                        in_=w1_d[l, :, co:co + 512].rearrange(
                            "(po pi) f -> pi po f", pi=P))
                    for ni in range(4):
                        fi = sl * 4 + ni
                        bc = PC_B1 + cq * 8 + fi
                        for (c0, c1) in CH512:
                            w = c1 - c0
                            ps = bank("psf1")
                            for k in range(KT):
                                nc.tensor.matmul(
                                    ps[:, :w], w1t[:, k, ni * P:(ni + 1) * P],
                                    hb[k][:, c0:c1],
                                    start=(k == 0), stop=(k == KT - 1))
                            nc.scalar.activation(gt[fi][:, c0:c1], ps[:, :w],
                                                 AF.Gelu, bias=pt[:, bc:bc + 1])
                for half in range(2):
                    w2t = wbfp.tile([P, KT, 512], bf16, tag="wbf", name="w2t")
                    nc.sync.dma_start(
                        out=w2t,
                        in_=w2_d[l, cq * 1024:(cq + 1) * 1024,
                                 half * 512:(half + 1) * 512].rearrange(
                            "(po pi) f -> pi po f", pi=P))
                    for ni in range(4):
                        n = half * 4 + ni
                        for (c0, c1) in CH512:
                            w = c1 - c0
                            ps = bank("psf2")
                            for k2 in range(KT):
                                nc.tensor.matmul(
                                    ps[:, :w], w2t[:, k2, ni * P:(ni + 1) * P],
                                    gt[k2][:, c0:c1],
                                    start=(k2 == 0), stop=(k2 == KT - 1))
                            sc = (pt[:, PC_B2 + n:PC_B2 + n + 1]
                                  if cq == 3 else 0.0)
                            nc.vector.scalar_tensor_tensor(
                                out=xt[n][:, c0:c1], in0=ps[:, :w], scalar=sc,
                                in1=xt[n][:, c0:c1], op0=OP.add, op1=OP.add)

        # ---------------- epilogue: ln_post + out proj ----------------
        gp = [postt[:, k:k + 1] for k in range(KT)]
        bp = [postt[:, 8 + k:8 + k + 1] for k in range(KT)]
        for s, off in ((0, 0), (1, 512)):
            hb = [hpool.tile([P, 256], bf16, tag=f"hp{k}", name=f"hp{k}")
                  for k in range(KT)]
            emit_ln(off, off + 256, gp, bp, hb, None, 0)
            pso = bank("psout")[0:VAE, :256]
            for k in range(KT):
                nc.tensor.matmul(pso, woutt[:, k * VAE:(k + 1) * VAE], hb[k],
                                 start=(k == 0), stop=(k == KT - 1))
            y = ypool.tile([VAE, 256], f32, tag="y", name="y")
            nc.vector.tensor_scalar_add(y, pso, boutt)
            nc.sync.dma_start(out=out_d[s].rearrange("r c -> c r"), in_=y)

    nc.finalize()
    return nc


def kernel(**inputs):
    global _PROG
    from concourse.bass_utils import run_bass_kernel_spmd
    in_maps, LB, perm = _host_prep(inputs)
    if LB not in _PROGS:
        _PROGS[LB] = _build_bass(LB)
    _PROG = _PROGS[LB]
    res = run_bass_kernel_spmd(_PROG, in_maps, list(range(NCORES)))
    out = np.zeros((B, M, VAE), np.float32)
    for c in range(NCORES):
        sa, sb = perm[c]
        out[sa] = res.results[c]["out"][0]
        out[sb] = res.results[c]["out"][1]
    return out



# revision 16
# speedup vs baseline: 2.0529x; 2.0529x over previous
"""Trainium2 Bass kernel for nn_Decoder_1D_Matryoshka (12-layer masked decoder).

Strategy: data-parallel over B (16 samples -> 8 cores x 2), with the two
samples per core merged into one token axis so every weight slab is loaded
once per layer. Residual kept transposed (features on partitions).

Attention exploits the mask structure without per-head mask multiplies:
  (1) latent queries never attend mask keys -> that score quadrant skipped;
  (2) inactive-key masking is per-key == per-PSUM-partition -> folded into
      the exp() bias column (additive -30);
  (3) the causal part is a *constant* 128x128 triangle on diagonal
      latent chunks (+ one per-sample 0/1 block), applied as small DVE muls;
  (4) softmax row-sums are merged into the AV matmuls via a ones-column
      appended to each head's V block (out partition 64 holds the sum);
  (5) one PSUM bank per head for AV+sum -> heads pipeline across banks.
LN gains/biases are folded into adjacent weight matrices host-side (exact),
so on-chip LN is only (x-mu)*rstd. q/k projections + LN statistics run in
fp8e4 DoubleRow (2x tensor throughput); v/proj/ff stay bf16.
Samples with Ni<=128 run with a 384-token sequence; host pairs one long
(512) and one short (384) sample per core when possible.
"""

import numpy as np
import ml_dtypes

B, NLAT, DIN = 16, 256, 32
D, H, NL, VAE = 1024, 16, 12, 16
M = 256
L = 512
DH = 64
NCORES = 8
BPC = 2
P = 128
KT = D // P       # 8 feature bands
FF = 4 * D

BF16 = ml_dtypes.bfloat16
E4 = ml_dtypes.float8_e4m3
F8MAX = np.float32(240.0)

# param-tile column layout (one [128, 112] f32 DMA per layer)
PC_SQ = 0      # 8: q dequant scale per band
PC_SK = 8      # 8
PC_BQ = 16     # 8: q bias (pre-scaled)
PC_BK = 24     # 8
PC_BP = 32     # 8: proj bias
PC_B1 = 40     # 32: fc1 bias
PC_B2 = 72     # 8: fc2 bias
PCOLS = 112


def _layernorm_np(x, g, b, eps=1e-5):
    mu = x.mean(-1, keepdims=True)
    var = ((x - mu) ** 2).mean(-1, keepdims=True)
    return (x - mu) / np.sqrt(var + eps) * g + b


def _quant_cols(w):
    """per-output-column absmax quantize to TRN e4m3 (max 240)."""
    amax = np.abs(w).max(axis=0, keepdims=True)
    s = np.where(amax > 0, amax / F8MAX, np.float32(1.0)).astype(np.float32)
    w8 = np.clip(w / s, -F8MAX, F8MAX).astype(E4)
    return w8, s[0]


def _host_prep(inputs):
    """Returns (in_maps, LB, perm) where perm[core] = (sampleA, sampleB)."""
    f32 = np.float32
    lat = inputs["latents"].astype(f32)
    x_lat = lat.reshape(B * NLAT, DIN) @ inputs["input_w"].astype(f32)
    x_lat = x_lat.reshape(B, NLAT, D) + inputs["input_b"][None, None, :]
    x_lat = x_lat + inputs["latents_pos_embed"][None, :, :]
    mt = inputs["mask_tokens"].reshape(1, 1, D) + inputs["pos_embed_full"]
    mt = np.broadcast_to(mt, (B, M, D))
    x = np.concatenate([mt, x_lat], axis=1)                       # (B, L, D)
    x = _layernorm_np(x, inputs["ln_pre_g"], inputs["ln_pre_b"]).astype(f32)

    ni = np.asarray(inputs["num_activated"]).astype(np.int64)

    # ---- sample pairing: big samples first, small last ----
    leff = np.where(ni <= M - P, 384, 512)   # Ni<=128 -> 384-token class
    order = np.argsort(-leff, kind="stable")
    LB = 384 if (leff[order[NCORES:]] == 384).all() else 512
    perm = [(int(order[i]), int(order[2 * NCORES - 1 - i])) for i in range(NCORES)]
    latB = LB - M
    NLATB = latB // P

    # ---- weights (fold LN g/b into adjacent mats: exact) ----
    scale = np.float32(DH ** -0.5)
    qkv = inputs["qkv_w"].astype(f32).copy()
    qkvb = inputs["qkv_b"].astype(f32).copy()
    ln1g = inputs["ln1_g"].astype(f32)
    ln1b = inputs["ln1_b"].astype(f32)
    for l in range(NL):
        qkvb[l] = qkvb[l] + ln1b[l] @ qkv[l]
        qkv[l] = qkv[l] * ln1g[l][:, None]
    fc1 = inputs["fc1_w"].astype(f32).copy()
    fc1b = inputs["fc1_b"].astype(f32).copy()
    ln2g = inputs["ln2_g"].astype(f32)
    ln2b = inputs["ln2_b"].astype(f32)
    for l in range(NL):
        fc1b[l] = fc1b[l] + ln2b[l] @ fc1[l]
        fc1[l] = fc1[l] * ln2g[l][:, None]
    wout = inputs["out_w"].astype(f32).copy()
    bout = inputs["out_b"].astype(f32) + inputs["ln_post_b"].astype(f32) @ wout
    wout = wout * inputs["ln_post_g"].astype(f32)[:, None]

    wqk8 = np.zeros((NL, D, 2 * D), E4)
    params = np.zeros((NL, P, PCOLS), f32)
    for l in range(NL):
        wqk = np.concatenate([qkv[l][:, :D] * scale, qkv[l][:, D:2 * D]], axis=1)
        w8, s = _quant_cols(wqk)
        wqk8[l] = w8
        params[l, :, PC_SQ:PC_SK + 8] = s.reshape(16, P).T
    bq = qkvb[:, :D] * scale
    bk = qkvb[:, D:2 * D]
    params[:, :, PC_BQ:PC_BQ + 8] = bq.reshape(NL, 8, P).transpose(0, 2, 1)
    params[:, :, PC_BK:PC_BK + 8] = bk.reshape(NL, 8, P).transpose(0, 2, 1)
    params[:, :, PC_BP:PC_BP + 8] = inputs["proj_b"].reshape(NL, 8, P).transpose(0, 2, 1)
    params[:, :, PC_B1:PC_B1 + 32] = fc1b.reshape(NL, 32, P).transpose(0, 2, 1)
    params[:, :, PC_B2:PC_B2 + 8] = inputs["fc2_b"].reshape(NL, 8, P).transpose(0, 2, 1)

    bvrow = qkvb[:, 2 * D:].reshape(NL, 1, D).astype(f32)

    postparams = np.zeros((P, 16), f32)  # unused g/b slots kept for layout

    # repack q/k weights so each 128-col n-tile has its 8 contraction planes
    # adjacent in SBUF (dual-fp8 LdWeights needs contiguous plane pairs):
    # [l, row(po*128+pi), n*128+c] -> [l, n, pi, po, c]
    wqk8_r = np.ascontiguousarray(
        wqk8.reshape(NL, KT, P, 16, P).transpose(0, 3, 2, 1, 4))

    # constant causal triangle for diagonal latent chunks: tri[k,q] = k<=q
    kk = np.arange(P)
    tri = (kk[:, None] <= kk[None, :]).astype(BF16)               # (128,128)

    static = {
        "wqk8": wqk8_r,
        "wv": np.ascontiguousarray(qkv[:, :, 2 * D:].astype(BF16)),
        "wproj": np.ascontiguousarray(inputs["proj_w"].astype(BF16)),
        "w1": np.ascontiguousarray(fc1.astype(BF16)),
        "w2": np.ascontiguousarray(inputs["fc2_w"].astype(BF16)),
        "params": params,
        "bvrow": np.ascontiguousarray(bvrow),
        "postparams": postparams,
        "wout": np.ascontiguousarray(wout.astype(BF16).reshape(KT, P, VAE)),
        "bout": bout.astype(f32).reshape(VAE, 1),
        "mtri": np.ascontiguousarray(tri),
        "v2init": None,  # filled below (needs LB)
    }

    def actbias(n_act, nchunk):
        # exp bias per latent-key chunk: -2 - 30*(key inactive)
        cols = np.zeros((P, nchunk), f32)
        for j in range(nchunk):
            act = (np.arange(P) + P * j) < n_act
            cols[:, j] = np.where(act, -2.0, -32.0)
        return cols

    def offdiag(n_act):
        # chunk-1 mask for queries [chunk0 | chunk1-diag]:
        # cols 0:128 -> 0 if Ni>128 (chunk0 queries all active, causal kills)
        # cols 128:256 -> constant triangle
        m = np.ones((P, 2 * P), f32)
        if n_act > P:
            m[:, :P] = 0.0
        m[:, P:] = tri.astype(f32)
        return m.astype(BF16)

    NMB_h = (512 + LB) // P
    v2init = np.zeros((P, NMB_h, 16, P), BF16)
    v2init[:, :, :, 0] = BF16(1.0)
    static["v2init"] = np.ascontiguousarray(v2init)
    in_maps = []
    for c in range(NCORES):
        sa, sb = perm[c]
        xc = np.concatenate([x[sa], x[sb, :LB]], axis=0)          # (T, D)
        x0T = np.ascontiguousarray(xc.T).astype(f32)              # (D, T)
        m = {
            "x0T": x0T,
            "actbA": np.ascontiguousarray(actbias(ni[sa], 2)),
            "actbB": np.ascontiguousarray(actbias(ni[sb], NLATB)),
            "mA1": np.ascontiguousarray(offdiag(ni[sa])),
            "mB1": np.ascontiguousarray(offdiag(ni[sb]) if NLATB == 2
                                        else np.zeros((P, 2 * P), BF16)),
        }
        m.update(static)
        in_maps.append(m)
    return in_maps, LB, perm


_PROGS = {}
_PROG = None


def _build_bass(LB, debug=False):
    import concourse.bass as bass
    import concourse.bacc as bacc
    import concourse.tile as tile
    import concourse.mybir as mybir
    from contextlib import ExitStack

    f32 = mybir.dt.float32
    bf16 = mybir.dt.bfloat16
    fp8 = mybir.dt.float8e4
    AF = mybir.ActivationFunctionType
    OP = mybir.AluOpType
    DR = mybir.MatmulPerfMode.DoubleRow

    T = 512 + LB
    latB = LB - M                 # latent tokens of sample B (128 or 256)
    NLATB = latB // P             # latent key chunks of B (1 or 2)
    NMB = T // P                  # 128-token chunks (7 or 8)
    CH512 = [(0, 512), (512, T)]

    nc = bacc.Bacc(None, target_bir_lowering=False, debug=False)
    dp = nc.declare_dram_parameter
    x0T = dp("x0T", [D, T], f32, isOutput=False)
    actbA_d = dp("actbA", [P, 2], f32, isOutput=False)
    actbB_d = dp("actbB", [P, NLATB], f32, isOutput=False)
    mA1_d = dp("mA1", [P, 2 * P], bf16, isOutput=False)
    mB1_d = dp("mB1", [P, 2 * P], bf16, isOutput=False)
    mtri_d = dp("mtri", [P, P], bf16, isOutput=False)
    v2init_d = dp("v2init", [P, NMB, 16, P], bf16, isOutput=False)
    wqk8_d = dp("wqk8", [NL, 16, P, KT, P], fp8, isOutput=False)
    wv_d = dp("wv", [NL, D, D], bf16, isOutput=False)
    wproj_d = dp("wproj", [NL, D, D], bf16, isOutput=False)
    w1_d = dp("w1", [NL, D, FF], bf16, isOutput=False)
    w2_d = dp("w2", [NL, FF, D], bf16, isOutput=False)
    params_d = dp("params", [NL, P, PCOLS], f32, isOutput=False)
    bvrow_d = dp("bvrow", [NL, 1, D], f32, isOutput=False)
    postparams = dp("postparams", [P, 16], f32, isOutput=False)
    wout_d = dp("wout", [KT, P, VAE], bf16, isOutput=False)
    bout_d = dp("bout", [VAE, 1], f32, isOutput=False)
    out_d = dp("out", [BPC, M, VAE], f32, isOutput=True)
    if debug:
        dbg_v2 = dp("dbg_v2", [P, NMB, 16, P], bf16, isOutput=True)
        dbg_at = dp("dbg_at", [2, P, T], bf16, isOutput=True)
        dbg_q = dp("dbg_q", [P, T], fp8, isOutput=True)
        dbg_k = dp("dbg_k", [P, T], fp8, isOutput=True)
        dbg_h = dp("dbg_h", [P, T], bf16, isOutput=True)
        dbg_x = dp("dbg_x", [P, T], f32, isOutput=True)

    def rchunks(c0, c1):
        out = []
        c = c0
        while c < c1:
            out.append((c, min(c + 256, c1)))
            c = out[-1][1]
        return out

    with ExitStack() as ctx:
        tc = ctx.enter_context(tile.TileContext(nc))
        # ---- pools ----
        persist = ctx.enter_context(tc.tile_pool(name="persist", bufs=1))
        wqkp = ctx.enter_context(tc.tile_pool(name="wqk", bufs=6))
        wbfp = ctx.enter_context(tc.tile_pool(name="wbf", bufs=3))
        hpool = ctx.enter_context(tc.tile_pool(name="h", bufs=1))
        tpool = ctx.enter_context(tc.tile_pool(name="lnt", bufs=2))
        h8pool = ctx.enter_context(tc.tile_pool(name="h8", bufs=1))
        x8pool = ctx.enter_context(tc.tile_pool(name="x8", bufs=2))
        gpool = ctx.enter_context(tc.tile_pool(name="g", bufs=1))
        epool = ctx.enter_context(tc.tile_pool(name="e", bufs=2))
        spool = ctx.enter_context(tc.tile_pool(name="stats", bufs=4))
        rpool = ctx.enter_context(tc.tile_pool(name="rsb", bufs=2))
        ppool = ctx.enter_context(tc.tile_pool(name="lparams", bufs=2))
        ypool = ctx.enter_context(tc.tile_pool(name="yout", bufs=2))
        psp = ctx.enter_context(tc.tile_pool(name="psp", bufs=8, space="PSUM"))

        def bank(name):
            return psp.tile([P, 512], f32, tag="bank", name=name)

        # ---- persistent tiles ----
        xt = [persist.tile([P, T], f32, tag=f"x{k}", name=f"x{k}") for k in range(KT)]
        qt = [persist.tile([P, T], fp8, tag=f"q{n}", name=f"q{n}") for n in range(KT)]
        kt_ = [persist.tile([P, T], fp8, tag=f"k{n}", name=f"k{n}") for n in range(KT)]
        v2 = persist.tile([P, NMB, 16, P], bf16, tag="v2", name="v2")
        attnT = [persist.tile([P, T], bf16, tag=f"at{k}", name=f"at{k}") for k in range(KT)]
        actbA = persist.tile([P, 2], f32, tag="actbA", name="actbA")
        actbB = persist.tile([P, NLATB], f32, tag="actbB", name="actbB")
        mA1 = persist.tile([P, 2 * P], bf16, tag="mA1", name="mA1")
        mB1 = persist.tile([P, 2 * P], bf16, tag="mB1", name="mB1")
        mtri = persist.tile([P, P], bf16, tag="mtri", name="mtri")
        btile = persist.tile([P, D], f32, tag="btile", name="btile")
        ones8 = persist.tile([P, 2, P], fp8, tag="ones8", name="ones8")
        ones_row = persist.tile([1, P], bf16, tag="ones_row", name="ones_row")
        woutt = persist.tile([P, KT * VAE], bf16, tag="wo", name="wo")
        boutt = persist.tile([VAE, 1], f32, tag="bout", name="bout")
        postt = persist.tile([P, 16], f32, tag="post", name="post")
        epst = persist.tile([1, 1], f32, tag="eps", name="eps")
        neg2 = persist.tile([P, 1], f32, tag="neg2", name="neg2")

        nc.vector.memset(epst, 1e-5)
        nc.vector.memset(neg2, -2.0)
        nc.vector.memset(ones8, 1.0)
        nc.vector.memset(ones_row, 1.0)

        for k in range(KT):
            nc.sync.dma_start(out=xt[k], in_=x0T[k * P:(k + 1) * P, :])
            nc.sync.dma_start(out=woutt[:, k * VAE:(k + 1) * VAE], in_=wout_d[k])
        nc.sync.dma_start(out=actbA, in_=actbA_d[:, :])
        nc.sync.dma_start(out=actbB, in_=actbB_d[:, :])
        nc.sync.dma_start(out=mA1, in_=mA1_d[:, :])
        nc.sync.dma_start(out=mB1, in_=mB1_d[:, :])
        nc.sync.dma_start(out=mtri, in_=mtri_d[:, :])
        nc.sync.dma_start(out=v2, in_=v2init_d[:, :, :, :])
        nc.sync.dma_start(out=boutt, in_=bout_d[:, :])
        nc.sync.dma_start(out=postt, in_=postparams[:, :])

        def emit_ln(c0, c1, out_tiles, out8, ow):
            """LN over features of xt[:, c0:c1] -> out_tiles[k][:, ow:ow+w]
            (bf16), optionally also into out8 [P,8,T] fp8 (gpsimd copy).
            Stats via fp8 DoubleRow pairs; psx/psxx share banks (col halves).
            Normalization: two ones-broadcast matmuls (prs, pms = mu*rstd),
            then per band out = x*prs - pms (2 vector ops, bf16 out)."""
            w = c1 - c0
            rcs = rchunks(c0, c1)
            sbank = [bank("psstat") for _ in rcs]
            for jp in range(4):
                xq = x8pool.tile([P, 2, 512], fp8, tag="xq", name="xq")
                xsq = x8pool.tile([P, 2, 512], fp8, tag="xsq", name="xsq")
                for i in range(2):
                    nc.vector.tensor_scalar_mul(
                        xq[:, i, :w], xt[2 * jp + i][:, c0:c1], 0.25)
                nc.vector.tensor_mul(xsq[:, :, :w], xq[:, :, :w], xq[:, :, :w])
                for ri, (r0, r1) in enumerate(rcs):
                    rw = r1 - r0
                    o = r0 - c0
                    # one start on first MM into the bank (clears whole-bank
                    # has_written), one stop on the very last; the co-packed
                    # sum/sumsq groups then accumulate per-element correctly.
                    nc.tensor.matmul(sbank[ri][:, :rw], ones8, xq[:, :, o:o + rw],
                                     start=(jp == 0), stop=False, perf_mode=DR,
                                     skip_group_check=True)
                    nc.tensor.matmul(sbank[ri][:, 256:256 + rw], ones8,
                                     xsq[:, :, o:o + rw],
                                     start=False, stop=(jp == 3), perf_mode=DR,
                                     skip_group_check=True)
            mu = spool.tile([1, 512], f32, tag="stat", name="mu")
            e2 = spool.tile([1, 512], f32, tag="stat", name="e2")
            rstd = spool.tile([1, 512], f32, tag="stat", name="rstd")
            musq = spool.tile([1, 512], f32, tag="stat", name="musq")
            mursb = spool.tile([1, 512], bf16, tag="statb", bufs=2, name="mursb")
            rstdb = spool.tile([1, 512], bf16, tag="statb", bufs=2, name="rstdb")
            for ri, (r0, r1) in enumerate(rcs):
                rw = r1 - r0
                o = r0 - c0
                nc.vector.tensor_scalar_mul(mu[:, o:o + rw], sbank[ri][0:1, :rw], 4.0 / D)
                nc.vector.tensor_scalar_mul(e2[:, o:o + rw],
                                            sbank[ri][0:1, 256:256 + rw], 16.0 / D)
            nc.vector.tensor_mul(musq[:, :w], mu[:, :w], mu[:, :w])
            nc.vector.tensor_sub(e2[:, :w], e2[:, :w], musq[:, :w])
            nc.scalar.activation(e2[:, :w], e2[:, :w], AF.Sqrt, bias=epst)
            nc.vector.reciprocal_approx_fast(out=rstd[:, :w], in_=e2[:, :w])
            nc.vector.tensor_mul(musq[:, :w], mu[:, :w], rstd[:, :w])
            nc.vector.tensor_copy(out=mursb[:, :w], in_=musq[:, :w])
            nc.vector.tensor_copy(out=rstdb[:, :w], in_=rstd[:, :w])
            pms = bank("pms")
            prs = bank("prs")
            nc.tensor.matmul(pms[:, :w], ones_row, mursb[:, :w], start=True, stop=True)
            nc.tensor.matmul(prs[:, :w], ones_row, rstdb[:, :w], start=True, stop=True)
            for k in range(KT):
                t = tpool.tile([P, 512], f32, tag="lnt", name="lnt")
                nc.vector.tensor_mul(t[:, :w], xt[k][:, c0:c1], prs[:, :w])
                nc.vector.tensor_sub(out_tiles[k][:, ow + 0:ow + w], t[:, :w],
                                     pms[:, :w])
                if out8 is not None:
                    nc.gpsimd.tensor_copy(out=out8[:, k, ow:ow + w],
                                          in_=out_tiles[k][:, ow:ow + w])

        def attn_head(hh, off, SL, nlat, voff, actb, mdiag1):
            """Attention for head hh of one sample (token offset off, seq SL).
            voff: first v2 chunk of the sample. actb: exp-bias columns
            [P, nlat]. mdiag1: [P,256] mask for chunk-1 (None if nlat==1)."""
            band = hh // 2
            po = (hh % 2) * DH
            q_ap = qt[band]
            k_ap = kt_[band]
            lw = SL - 256
            # mask-key chunks: only the 256 mask queries attend them.
            e01 = epool.tile([P, 2, 256], bf16, tag="e01", name="e01")
            psS01 = bank("psS01")
            for j in range(2):
                nc.tensor.matmul(psS01[:, j * 256:(j + 1) * 256],
                                 k_ap[po:po + DH, off + j * P:off + (j + 1) * P],
                                 q_ap[po:po + DH, off:off + 256],
                                 start=(j == 0), stop=(j == 1),
                                 skip_group_check=True)
            nc.scalar.activation(e01[:, :, :], psS01, AF.Exp, bias=neg2)
            # latent-key chunks: all SL queries; key-inactive mask folded
            # into the exp bias column; causal triangle applied after.
            elat = []
            for j in range(nlat):
                psS = bank("psSL")
                nc.tensor.matmul(psS[:, :SL],
                                 k_ap[po:po + DH, off + (2 + j) * P:off + (3 + j) * P],
                                 q_ap[po:po + DH, off:off + SL],
                                 start=True, stop=True)
                e = epool.tile([P, 512], bf16, tag=f"el{j}", name=f"el{j}")
                nc.scalar.activation(e[:, :SL], psS[:, :SL], AF.Exp,
                                     bias=actb[:, j:j + 1])
                if j == 0:
                    nc.vector.tensor_mul(e[:, 256:384], e[:, 256:384], mtri)
                else:
                    nc.vector.tensor_mul(e[:, 256:256 + lw], e[:, 256:256 + lw],
                                         mdiag1[:, :lw])
                elat.append(e)
            # single AV+rowsum bank: v2 head block = [1, 63*0, v(64)], so
            # partition 0 = softmax sum (custom-DVE recip silently reads
            # partition 0 whatever the AP says, so the sum MUST live there),
            # partitions 64:128 = AV (HW-probe-proven base-64 DVE reads).
            psO = bank("psO")
            i = 0
            for j in range(2):
                nc.tensor.matmul(psO[0:P, 0:256],
                                 v2[:, voff + j, hh, :], e01[:, j, :],
                                 start=(i == 0), stop=False,
                                 skip_group_check=True)
                i += 1
            for j in range(nlat):
                nc.tensor.matmul(psO[0:P, 0:256],
                                 v2[:, voff + 2 + j, hh, :], elat[j][:, :256],
                                 start=False, stop=False,
                                 skip_group_check=True)
            for j in range(nlat):
                nc.tensor.matmul(psO[0:P, 256:256 + lw],
                                 v2[:, voff + 2 + j, hh, :], elat[j][:, 256:SL],
                                 start=False, stop=(j == nlat - 1),
                                 skip_group_check=True)
            # ones-column is FIRST in each v2 head block, so the softmax sum
            # lands on PSUM partition 0 (custom-DVE recip requires base 0;
            # reading at base partition 64 silently reads partition 0 on HW).
            rs = spool.tile([1, 512], f32, tag="rs", bufs=3, name="rs")
            nc.vector.reciprocal_approx_fast(out=rs[:, :SL],
                                             in_=psO[0:1, :SL])
            rsb = rpool.tile([DH, 512], f32, tag="rsb", name="rsb")
            nc.gpsimd.partition_broadcast(rsb[:, :SL], rs[:, :SL])
            nc.vector.tensor_mul(attnT[band][po:po + DH, off:off + SL],
                                 psO[64:128, :SL], rsb[:, :SL])

        for l in range(NL):
            pt = ppool.tile([P, PCOLS], f32, tag="pt", name="pt")
            bvr = ppool.tile([1, D], f32, tag="bvr", bufs=1, name="bvr")
            nc.sync.dma_start(out=pt, in_=params_d[l])
            nc.sync.dma_start(out=bvr, in_=bvrow_d[l])
            nc.gpsimd.partition_broadcast(btile, bvr)

            # ---------------- LN1 ----------------
            hb = [hpool.tile([P, T], bf16, tag=f"h{k}", name=f"h{k}") for k in range(KT)]
            h8 = h8pool.tile([P, KT, T], fp8, tag="h8", name="h8")
            for (c0, c1) in CH512:
                emit_ln(c0, c1, hb, h8, c0)

            # ---------------- q/k (fp8 DoubleRow) ----------------
            # interleave q and k bands so attention heads unlock early
            for idx in range(16):
                n = (idx // 2) + 8 * (idx % 2)
                wqk = wqkp.tile([P, KT, P], fp8, tag="wqk", name="wqk")
                nc.sync.dma_start(out=wqk, in_=wqk8_d[l, n])
                if n < 8:
                    dst, sc, bc = qt[n], PC_SQ + n, PC_BQ + n
                else:
                    dst, sc, bc = kt_[n - 8], PC_SK + n - 8, PC_BK + n - 8
                for (C0, C1) in CH512:
                    ps = bank("psqk")
                    for ri, (r0, r1) in enumerate(rchunks(C0, C1)):
                        rw = r1 - r0
                        o = r0 - C0
                        for j in range(0, KT, 2):
                            nc.tensor.matmul(
                                ps[:, o:o + rw], wqk[:, j:j + 2, :],
                                h8[:, j:j + 2, r0:r1],
                                start=(ri == 0 and j == 0),
                                stop=(j == KT - 2), perf_mode=DR,
                                skip_group_check=True)
                    nc.vector.tensor_scalar(
                        out=dst[:, C0:C1], in0=ps[:, :C1 - C0],
                        scalar1=pt[:, sc:sc + 1], scalar2=pt[:, bc:bc + 1],
                        op0=OP.mult, op1=OP.add)

            # ---------------- v (bf16, h-stationary) ----------------
            for half in range(2):
                wv = wbfp.tile([P, KT, 512], bf16, tag="wbf", name="wv")
                nc.sync.dma_start(
                    out=wv,
                    in_=wv_d[l, :, half * 512:(half + 1) * 512].rearrange(
                        "(po pi) f -> pi po f", pi=P))
                for mb in range(NMB):
                    ps = bank("psv")
                    for k in range(KT):
                        nc.tensor.matmul(ps, hb[k][:, mb * P:(mb + 1) * P],
                                         wv[:, k, :], start=(k == 0), stop=(k == KT - 1))
                    nc.vector.tensor_add(
                        v2[:, mb, half * 8:(half + 1) * 8, 64:128], ps,
                        btile[:, half * 512:(half + 1) * 512])

            # ---------------- attention (A/B interleaved) ----------------
            for hh in range(H):
                attn_head(hh, 0, 512, 2, 0, actbA, mA1)
                attn_head(hh, 512, LB, NLATB, 4, actbB,
                          mB1 if NLATB == 2 else None)

            if debug and l == 0:
                nc.sync.dma_start(out=dbg_v2[:, :, :, :], in_=v2)
                nc.sync.dma_start(out=dbg_at[0, :, :], in_=attnT[0])
                nc.sync.dma_start(out=dbg_at[1, :, :], in_=attnT[7])
                nc.sync.dma_start(out=dbg_q[:, :], in_=qt[0])
                nc.sync.dma_start(out=dbg_k[:, :], in_=kt_[0])
                nc.sync.dma_start(out=dbg_h[:, :], in_=hb[0])

            # ---------------- proj (+residual) ----------------
            for half in range(2):
                wp = wbfp.tile([P, KT, 512], bf16, tag="wbf", name="wp")
                nc.sync.dma_start(
                    out=wp,
                    in_=wproj_d[l, :, half * 512:(half + 1) * 512].rearrange(
                        "(po pi) f -> pi po f", pi=P))
                for ni in range(4):
                    n = half * 4 + ni
                    for (c0, c1) in CH512:
                        w = c1 - c0
                        ps = bank("pspj")
                        for k in range(KT):
                            nc.tensor.matmul(ps[:, :w], wp[:, k, ni * P:(ni + 1) * P],
                                             attnT[k][:, c0:c1],
                                             start=(k == 0), stop=(k == KT - 1))
                        nc.vector.scalar_tensor_tensor(
                            out=xt[n][:, c0:c1], in0=ps[:, :w],
                            scalar=pt[:, PC_BP + n:PC_BP + n + 1],
                            in1=xt[n][:, c0:c1], op0=OP.add, op1=OP.add)

            # ---------------- LN2 ----------------
            hb = [hpool.tile([P, T], bf16, tag=f"h{k}", name=f"h{k}") for k in range(KT)]
            for (c0, c1) in CH512:
                emit_ln(c0, c1, hb, None, c0)

            # ---------------- FF (4 quarters of d_ff) ----------------
            for cq in range(4):
                gt = [gpool.tile([P, T], bf16, tag=f"g{i}", name=f"g{i}")
                      for i in range(8)]
                for sl in range(2):
                    w1t = wbfp.tile([P, KT, 512], bf16, tag="wbf", name="w1t")
                    co = cq * 1024 + sl * 512
                    nc.sync.dma_start(
                        out=w1t,
                        in_=w1_d[l, :, co:co + 512].rearrange(
                            "(po pi) f -> pi po f", pi=P))
                    for ni in range(4):
                        fi = sl * 4 + ni
                        bc = PC_B1 + cq * 8 + fi
                        for (c0, c1) in CH512:
                            w = c1 - c0
                            ps = bank("psf1")
                            for k in range(KT):
                                nc.tensor.matmul(
                                    ps[:, :w], w1t[:, k, ni * P:(ni + 1) * P],
                                    hb[k][:, c0:c1],
                                    start=(k == 0), stop=(k == KT - 1))
                            nc.scalar.activation(gt[fi][:, c0:c1], ps[:, :w],
                                                 AF.Gelu, bias=pt[:, bc:bc + 1])
                for half in range(2):
                    w2t = wbfp.tile([P, KT, 512], bf16, tag="wbf", name="w2t")
                    nc.sync.dma_start(
                        out=w2t,
                        in_=w2_d[l, cq * 1024:(cq + 1) * 1024,
                                 half * 512:(half + 1) * 512].rearrange(
                            "(po pi) f -> pi po f", pi=P))
                    for ni in range(4):
                        n = half * 4 + ni
                        for (c0, c1) in CH512:
                            w = c1 - c0
                            ps = bank("psf2")
                            for k2 in range(KT):
                                nc.tensor.matmul(
                                    ps[:, :w], w2t[:, k2, ni * P:(ni + 1) * P],
                                    gt[k2][:, c0:c1],
                                    start=(k2 == 0), stop=(k2 == KT - 1))
                            sc = (pt[:, PC_B2 + n:PC_B2 + n + 1]
                                  if cq == 3 else 0.0)
                            nc.vector.scalar_tensor_tensor(
                                out=xt[n][:, c0:c1], in0=ps[:, :w], scalar=sc,
                                in1=xt[n][:, c0:c1], op0=OP.add, op1=OP.add)

            if debug and l == 0:
                nc.sync.dma_start(out=dbg_x[:, :], in_=xt[0])

        # ---------------- epilogue: ln_post + out proj ----------------
        for s, off in ((0, 0), (1, 512)):
            hb = [hpool.tile([P, 256], bf16, tag=f"hp{k}", name=f"hp{k}")
                  for k in range(KT)]
            emit_ln(off, off + 256, hb, None, 0)
            pso = bank("psout")[0:VAE, :256]
            for k in range(KT):
                nc.tensor.matmul(pso, woutt[:, k * VAE:(k + 1) * VAE], hb[k],
                                 start=(k == 0), stop=(k == KT - 1))
            y = ypool.tile([VAE, 256], f32, tag="y", name="y")
            nc.vector.tensor_scalar_add(y, pso, boutt)
            nc.sync.dma_start(out=out_d[s].rearrange("r c -> c r"), in_=y)

    nc.finalize()
    return nc


def kernel(**inputs):
    global _PROG
    from concourse.bass_utils import run_bass_kernel_spmd
    in_maps, LB, perm = _host_prep(inputs)
    if LB not in _PROGS:
        _PROGS[LB] = _build_bass(LB)
    _PROG = _PROGS[LB]
    res = run_bass_kernel_spmd(_PROG, in_maps, list(range(NCORES)))
    out = np.zeros((B, M, VAE), np.float32)
    for c in range(NCORES):
        sa, sb = perm[c]
        out[sa] = res.results[c]["out"][0]
        out[sb] = res.results[c]["out"][1]
    return out
